# revision 27
# baseline (speedup 1.0000x reference)
"""GCN message-passing kernel for Trainium2 (8 NeuronCores, Bass/Tile).

out = coef * relu(C_U * D^-1/2 A~^T D^-1/2 (x W^T + b)),  A~ = A + I

Strategy (dst-sharded, fully static SPMD program):
- Core c owns a 12,500-node dst range. Host deals dsts into 64-wide
  "windows" (degree-sorted round-robin for load balance), buckets each
  core's edges by (window, src-segment), pads to static per-(w,seg)
  quotas (max over cores/windows) so all 8 cores run one program.
- W commutes with aggregation: aggregate xs = dis_src*x rows first,
  apply W once per output node afterwards.
- Device: dma_gather pulls x rows (512B) from HBM by int16 index
  (4 src segments of 32768 rows + a per-core self-loop table); ACT
  casts msgs fp32->fp16; DVE builds per-pass "value-hot" [e,64]
  matrices (iota==dstoff)*dis_src in fp16; PE contracts
  msgs^T @ vh into PSUM [128=D, 64=dst] per window; stage-2 matmul
  applies W^T; ACT fuses relu + coef*C_U*dis_dst scale; DMA out.
- Host unpermutes the window-ordered output rows.
"""

import sys
import types

import numpy as np


def _install_ntff_hook_bridge():
    """antenv.axon_hooks is missing from this image; bridge it so
    run_bass_kernel_spmd(trace=True) can profile. Harmless if unused."""
    if "antenv.axon_hooks" in sys.modules:
        return
    hooks = types.ModuleType("antenv.axon_hooks")
    hooks._HOOK = None

    def _get():
        if hooks._HOOK is None:
            try:
                from trn_agent_boot.trn_boot import _ntff_profile_via_ctypes

                hooks._HOOK = _ntff_profile_via_ctypes("/opt/axon/libaxon_pjrt.so")
            except Exception:
                hooks._HOOK = None
        return hooks._HOOK

    hooks.get_axon_ntff_profile_hook = _get
    hooks.set_axon_ntff_profile_hook = lambda h: setattr(hooks, "_HOOK", h)
    sys.modules["antenv.axon_hooks"] = hooks


_install_ntff_hook_bridge()

C_SIGMA = 2.0
C_U = 1.0
SEG = 32768  # dma_gather int16 index reach
W_WIN = 64  # dst window width (one-hot width)
N_CORES = 8


def _ceil(a, b):
    return (a + b - 1) // b


def _wrap16(idx, ncols):
    """[n] int16 -> [128, ncols] wrapped in 16 partitions, replicated x8."""
    n = idx.shape[0]
    out = np.zeros((16, ncols), dtype=np.int16)
    out[np.arange(n) % 16, np.arange(n) // 16] = idx
    return np.tile(out, (8, 1))


def _slotwrap(vals, ncols, dtype):
    """[n] -> [128, ncols] with slot i at [i%128, i//128]."""
    n = vals.shape[0]
    out = np.zeros((128, ncols), dtype=dtype)
    out[np.arange(n) % 128, np.arange(n) // 128] = vals
    return out


class _Prep:
    """Host-side sharding/preprocessing result."""


def prepare(x, edge_index, W, b, n_cores=N_CORES, w_win=W_WIN, group=8):
    f16 = np.float16
    N, D = x.shape
    assert N % n_cores == 0
    npc = N // n_cores
    nwin = _ceil(npc, w_win)
    # make nwin a multiple of 4 so groups of 8 tile as 8,...,8,4
    nwin = _ceil(nwin, 4) * 4
    nseg = _ceil(N, SEG)

    src = np.asarray(edge_index[0], dtype=np.int64)
    dst = np.asarray(edge_index[1], dtype=np.int64)
    deg = np.bincount(src, minlength=N).astype(np.float32) + 1.0
    dis = deg ** -0.5  # float32

    p = _Prep()
    p.N, p.D, p.npc, p.nwin, p.nseg = N, D, npc, nwin, nseg
    p.n_cores, p.w_win, p.group = n_cores, w_win, group
    p.coef = np.sqrt(C_SIGMA / D).astype(np.float32)

    core_of = dst // npc
    dstloc = dst - core_of * npc

    # --- per-core window assignment: deal dsts (sorted by degree desc)
    # round-robin over nwin windows -> balanced loads; permutation undone
    # on the host at the end.
    # per-(node, segment) in-degree for balanced window packing
    segdeg = np.zeros((N, nseg), dtype=np.int64)
    np.add.at(segdeg, (dst, src // SEG), 1)

    p.win_members = []  # per core: [nwin*w_win] node-local ids (or -1 pad)
    p.win_of = np.empty((n_cores, npc), dtype=np.int32)
    p.pos_of = np.empty((n_cores, npc), dtype=np.int32)
    for c in range(n_cores):
        sd = segdeg[c * npc : (c + 1) * npc]  # [npc, nseg]
        tot = sd.sum(axis=1)
        order = np.argsort(-tot, kind="stable").astype(np.int32)
        # round-based LPT: each round deals one dst to every window,
        # heaviest remaining dst -> currently-lightest window (by max
        # per-segment load). Keeps window sizes exact.
        loads = np.zeros((nwin, nseg), dtype=np.int64)
        memb = -np.ones(nwin * w_win, dtype=np.int64)
        for r in range(w_win):
            chunk = order[r * nwin : (r + 1) * nwin]
            if len(chunk) == 0:
                break
            wsort = np.argsort(loads.max(axis=1), kind="stable")[: len(chunk)]
            p.win_of[c, chunk] = wsort
            p.pos_of[c, chunk] = r
            memb[wsort * w_win + r] = chunk
            loads[wsort] += sd[chunk]
        p.win_members.append(memb)

    # --- per-edge: core, window, dstoff, segment
    e_w = p.win_of[core_of, dstloc]
    e_off = p.pos_of[core_of, dstloc]
    e_q = (src // SEG).astype(np.int64)

    # --- quotas: max over (core, window) per segment, rounded to 32
    cnt = np.zeros((n_cores, nwin, nseg), dtype=np.int64)
    np.add.at(cnt, (core_of, e_w, e_q), 1)
    quotas = [max(32, int(_ceil(cnt[:, :, q].max(), 32) * 32)) for q in range(nseg)]
    p.quotas = quotas + [w_win]  # last stream = self loops
    p.nstream = nseg + 1
    p.cnt = cnt

    # --- static pass schedule (same for all cores)
    # stream q slots: window w occupies [w*Q, (w+1)*Q); 128-slot columns;
    # a column overlapping two windows is processed once per window.
    p.passes = []  # list of (w, q, col) in window-major emission order
    p.win_passes = [[] for _ in range(nwin)]  # per window: pass indices
    for w in range(nwin):
        for q in range(p.nstream):
            Q = p.quotas[q]
            c0 = (w * Q) // 128
            c1 = ((w + 1) * Q - 1) // 128
            for col in range(c0, c1 + 1):
                p.win_passes[w].append(len(p.passes))
                p.passes.append((w, q, col))
    p.npass = len(p.passes)

    # group layout
    p.ngroups = _ceil(nwin, group)
    p.group_sizes = [min(group, nwin - g * group) for g in range(p.ngroups)]
    assert all(gs in (group, 4) or gs == group for gs in p.group_sizes)

    # per-(group, stream): column count and pass count (static)
    p.gcols = [[gs * Q // 128 for Q in p.quotas] for gs in p.group_sizes]
    for row in p.gcols:
        for v in row:
            assert v * 128 % 128 == 0
    p.gpasses = []  # per group, per stream: number of passes
    for g, gs in enumerate(p.group_sizes):
        row = []
        for q in range(p.nstream):
            n = 0
            for w in range(g * group, g * group + gs):
                Q = p.quotas[q]
                n += ((w + 1) * Q - 1) // 128 - (w * Q) // 128 + 1
            row.append(n)
        p.gpasses.append(row)
    p.colsmax = max(max(row) for row in p.gpasses)

    # --- per-core slot fill + per-pass metadata
    p.idx_all = []  # [128, sum_g sum_q G*Q/16] int16
    p.doff_all = []  # [128, total_passes] f16
    p.disv_all = []  # [128, total_passes] f16
    p.sd = []
    tot_idx_cols = sum(
        sum(gs * Q // 16 for Q in p.quotas) for gs in p.group_sizes
    )
    tot_pass = sum(sum(row) for row in p.gpasses)
    p.tot_idx_cols, p.tot_pass = tot_idx_cols, tot_pass

    for c in range(n_cores):
        m = core_of == c
        cw, coff, cq, csrc = e_w[m], e_off[m], e_q[m], src[m]
        memb = p.win_members[c]

        # slot arrays per stream
        sl_idx = []
        sl_doff = []
        sl_dis = []
        for q in range(nseg):
            Q = p.quotas[q]
            S = nwin * Q
            nrows_q = min(N, (q + 1) * SEG) - q * SEG
            # spread pad reads across the segment (identical pad indices
            # serialize on one DRAM row: measured 3.6x slower)
            idx16 = ((np.arange(S, dtype=np.int64) * 7919) % nrows_q).astype(
                np.int16
            )
            doffv = -np.ones(S, dtype=np.float32)
            disv = np.zeros(S, dtype=np.float32)
            mq = cq == q
            wq, offq, srcq = cw[mq], coff[mq], csrc[mq]
            o = np.lexsort((srcq, wq))  # window-major, src-sorted within
            wq, offq, srcq = wq[o], offq[o], srcq[o]
            wcnt = np.bincount(wq, minlength=nwin)
            starts = np.concatenate([[0], np.cumsum(wcnt)[:-1]])
            rank = np.arange(len(wq)) - starts[wq]
            slot = wq * Q + rank
            idx16[slot] = (srcq - q * SEG).astype(np.int16)
            doffv[slot] = offq
            disv[slot] = dis[srcq]
            sl_idx.append(idx16)
            sl_doff.append(doffv)
            sl_dis.append(disv)
        # self stream: host pre-permutes the core's x slice into wrapped
        # window order -> plain sequential DMA on device (no gather).
        Q = w_win
        S = nwin * Q
        idx16 = np.zeros(S, dtype=np.int16)  # unused (sequential stream)
        doffv = -np.ones(S, dtype=np.float32)
        disv = np.zeros(S, dtype=np.float32)
        real = memb >= 0
        slots = np.arange(nwin * w_win)[real]
        nodes = memb[real]
        doffv[slots] = slots % w_win
        disv[slots] = dis[c * npc + nodes]
        sl_idx.append(idx16)
        sl_doff.append(doffv)
        sl_dis.append(disv)
        # wrapped layout [128, S/128, D]: slot i at [i%128, i//128]
        xsp = np.zeros((128, S // 128, D), dtype=np.float32)
        xsp[slots % 128, slots // 128] = x[c * npc + nodes]
        p.xself_perm = getattr(p, "xself_perm", [])
        p.xself_perm.append(xsp)

        # pack group-major: idx wrapped; passes built from slot arrays
        idx_cols = np.zeros((128, tot_idx_cols), dtype=np.int16)
        doff_cols = np.zeros((128, tot_pass), dtype=f16)
        disv_cols = np.zeros((128, tot_pass), dtype=f16)
        ic = 0
        pc_i = 0
        for g, gs in enumerate(p.group_sizes):
            w0 = g * group
            for q in range(p.nstream):
                Q = p.quotas[q]
                seg_slots = sl_idx[q][w0 * Q : (w0 + gs) * Q]
                ncol = gs * Q // 16
                idx_cols[:, ic : ic + ncol] = _wrap16(seg_slots, ncol)
                ic += ncol
            # stream-major within group (matches device vh-tile layout)
            for q in range(p.nstream):
                Q = p.quotas[q]
                for w in range(w0, w0 + gs):
                    c0 = (w * Q) // 128
                    c1 = ((w + 1) * Q - 1) // 128
                    for col in range(c0, c1 + 1):
                        s0 = col * 128
                        sl = np.arange(s0, s0 + 128)
                        inw = (sl >= w * Q) & (sl < (w + 1) * Q)
                        dv = np.full(128, -1.0, dtype=np.float32)
                        vv = np.zeros(128, dtype=np.float32)
                        gsl = sl[inw]
                        dv[inw] = sl_doff[q][gsl]
                        vv[inw] = sl_dis[q][gsl]
                        doff_cols[:, pc_i] = dv.astype(f16)
                        disv_cols[:, pc_i] = vv.astype(f16)
                        pc_i += 1
        assert pc_i == tot_pass, (pc_i, tot_pass)
        assert ic == tot_idx_cols
        p.idx_all.append(idx_cols)
        p.doff_all.append(doff_cols)
        p.disv_all.append(disv_cols)


        sdv = np.zeros((w_win, nwin), dtype=np.float32)
        nodes_per_win = memb.reshape(nwin, w_win)
        for w in range(nwin):
            mm = nodes_per_win[w] >= 0
            sdv[mm, w] = (
                p.coef * C_U * dis[c * npc + nodes_per_win[w][mm]]
            ).astype(np.float32)
        p.sd.append(sdv)

    # iota-expanded constant [128, w_win, colsmax] f16: value j at (p, j, c)
    io = np.broadcast_to(
        np.arange(w_win, dtype=np.float32)[None, :, None], (128, w_win, p.colsmax)
    )
    p.iota = np.ascontiguousarray(io.astype(f16))
    p.WT = np.ascontiguousarray(np.asarray(W, dtype=np.float32).T)
    p.b = np.asarray(b, dtype=np.float32)
    p.bias_nonzero = bool(np.any(p.b != 0))
    if p.bias_nonzero:
        # S_d = sum over incoming edges (incl self) of dis_src, laid out
        # as a row [1, nwin*w_win]: element w*w_win + i = window w, row i
        sb = np.zeros((n_cores, nwin * w_win), dtype=np.float32)
        np.add.at(sb, (core_of, e_w * w_win + e_off), dis[src])
        for c in range(n_cores):
            memb = p.win_members[c]
            real = memb >= 0
            slots = np.arange(nwin * w_win)[real]
            sb[c, slots] += dis[c * npc + memb[real]]
        p.sb = sb.reshape(n_cores, 1, nwin * w_win)
    return p


def build_program(p, debug_stage=5):
    import concourse.bacc as bacc
    import concourse.mybir as mybir
    import concourse.tile as tile

    f32, f16i, i16 = mybir.dt.float32, mybir.dt.float16, mybir.dt.int16
    D, nwin, group = p.D, p.nwin, p.group
    nstream = p.nstream

    nc = bacc.Bacc(
        "TRN2", target_bir_lowering=False, debug=False, num_swdge_queues=4
    )
    x_d = nc.dram_tensor("x", [p.N, D], f32, kind="ExternalInput")
    xself_d = nc.dram_tensor(
        "xself", [128, p.nwin * p.w_win // 128, D], f32, kind="ExternalInput"
    )
    wt_d = nc.dram_tensor("wt", [D, D], f32, kind="ExternalInput")
    iota_d = nc.dram_tensor("iota", [128, p.w_win, p.colsmax], f16i, kind="ExternalInput")
    idx_d = nc.dram_tensor("idx", [128, p.tot_idx_cols], i16, kind="ExternalInput")
    doff_d = nc.dram_tensor("doff", [128, p.tot_pass], f16i, kind="ExternalInput")
    disv_d = nc.dram_tensor("disv", [128, p.tot_pass], f16i, kind="ExternalInput")
    sd_d = nc.dram_tensor("sd", [p.w_win, nwin], f32, kind="ExternalInput")
    if p.bias_nonzero:
        sb_d = nc.dram_tensor("sb", [1, nwin * p.w_win], f32, kind="ExternalInput")
        b_d = nc.dram_tensor("b", [1, D], f32, kind="ExternalInput")
    # partition-major layout: out[i, w, :] = window w, row i (host unshards)
    out_d = nc.dram_tensor("out", [p.w_win, nwin, D], f32, kind="ExternalOutput")

    segs = []  # (dram AP, nrows) per gather stream
    for q in range(p.nseg):
        lo = q * SEG
        hi = min(p.N, lo + SEG)
        segs.append(x_d[lo:hi, :])
    segs.append(None)  # self stream: pre-permuted, sequential DMA

    with tile.TileContext(nc) as tc:
        with (
            tc.tile_pool(name="const", bufs=1) as constp,
            tc.tile_pool(name="meta", bufs=2) as metap,
            tc.tile_pool(name="gbuf", bufs=2) as gbufp,
            tc.tile_pool(name="msgs", bufs=2) as msgsp,
            tc.tile_pool(name="vh", bufs=2) as vhp,
            tc.tile_pool(name="aggx", bufs=3) as aggxp,
            tc.tile_pool(name="outsb", bufs=2) as outp,
            tc.tile_pool(name="ps1", bufs=4, space="PSUM") as ps1p,
            tc.tile_pool(name="ps2", bufs=2, space="PSUM") as ps2p,
        ):
            # constants
            wt32 = constp.tile([D, D], f32, tag="wt32")
            nc.sync.dma_start(wt32[:], wt_d[:])
            wt16 = constp.tile([D, D], f16i, tag="wt16")
            nc.scalar.copy(wt16[:], wt32[:])
            iota_sb = constp.tile([128, p.w_win, p.colsmax], f16i, tag="iota")
            nc.sync.dma_start(iota_sb[:], iota_d[:])
            sd_sb = constp.tile([p.w_win, nwin], f32, tag="sd")
            nc.sync.dma_start(sd_sb[:], sd_d[:])
            if p.bias_nonzero:
                sb_sb = constp.tile([1, nwin * p.w_win], f32, tag="sb")
                nc.sync.dma_start(sb_sb[:], sb_d[:])
                b32 = constp.tile([1, D], f32, tag="b32")
                nc.sync.dma_start(b32[:], b_d[:])
                b16 = constp.tile([1, D], f16i, tag="b16")
                nc.scalar.copy(b16[:], b32[:])
                sbrow16 = constp.tile([1, nwin * p.w_win], f16i, tag="sbw16")
                nc.scalar.copy(sbrow16[:], sb_sb[:])

            ic_base = 0
            pass_base = 0
            gq = [0]  # round-robin SWDGE queue counter (mutable closure)
            for g, gs in enumerate(p.group_sizes):
                w0 = g * group
                gidx_cols = sum(gs * Q // 16 for Q in p.quotas)
                gpass = sum(p.gpasses[g])
                idx_sb = metap.tile([128, gidx_cols], i16, tag="idx")
                nc.sync.dma_start(idx_sb[:], idx_d[:, ic_base : ic_base + gidx_cols])
                doff_sb = metap.tile([128, gpass], f16i, tag="doff")
                nc.sync.dma_start(doff_sb[:], doff_d[:, pass_base : pass_base + gpass])
                disv_sb = metap.tile([128, gpass], f16i, tag="disv")
                nc.sync.dma_start(disv_sb[:], disv_d[:, pass_base : pass_base + gpass])

                # gathers + cast + vh build, per stream
                gb = []
                ms = []
                vh = []
                icol = 0
                ppos = 0
                for q in range(nstream):
                    Q = p.quotas[q]
                    ncols = gs * Q // 128
                    nix = gs * Q // 16
                    npas = p.gpasses[g][q]
                    gt = gbufp.tile([128, ncols, D], f32, tag=f"g{q}")
                    if segs[q] is None:
                        # self stream: sequential slice of the pre-permuted
                        # per-core table
                        c0 = w0 * Q // 128
                        nc.sync.dma_start(gt[:], xself_d[:, c0 : c0 + ncols, :])
                    else:
                        # SWDGE ring holds 64 descs/lane -> max 1024 idxs
                        # per gather; round-robin the 4 queues (4 Q7 pairs
                        # generate in parallel).
                        off = 0
                        total = gs * Q
                        while off < total:
                            n = min(1024, total - off)
                            nc.gpsimd.dma_gather(
                                gt[:, off // 128 : (off + n) // 128, :],
                                segs[q],
                                idx_sb[:, icol + off // 16 : icol + (off + n) // 16],
                                n,
                                n,
                                D,
                                queue_num=gq[0] % 4,
                            )
                            gq[0] += 1
                            off += n
                    mt = msgsp.tile([128, ncols, D], f16i, tag=f"m{q}")
                    if debug_stage >= 2:
                        nc.scalar.copy(mt[:], gt[:])
                    vt = vhp.tile([128, p.w_win, npas], f16i, tag=f"v{q}")

                    def _bcast(ap2d, n=npas):
                        return ap2d.rearrange("p (o c) -> p o c", o=1).broadcast_to(
                            [128, p.w_win, n]
                        )

                    if debug_stage >= 3:
                        nc.vector.tensor_tensor(
                            vt[:],
                            iota_sb[:, :, :npas],
                            _bcast(doff_sb[:, ppos : ppos + npas]),
                            mybir.AluOpType.is_equal,
                        )
                        nc.vector.tensor_tensor(
                            vt[:],
                            vt[:],
                            _bcast(disv_sb[:, ppos : ppos + npas]),
                            mybir.AluOpType.mult,
                        )
                    gb.append(gt)
                    ms.append(mt)
                    vh.append(vt)
                    icol += nix
                    ppos += npas

                out_sb = outp.tile([p.w_win, gs, D], f32, tag="out")
                if debug_stage < 5:
                    nc.vector.memset(out_sb[:], 0.0)
                # windows
                pass_ctr = [0] * nstream
                for wl in range(gs):
                    w = w0 + wl
                    if debug_stage < 4:
                        continue
                    ps1 = ps1p.tile([D, p.w_win], f32, tag="ps1")
                    plist = p.win_passes[w]
                    for k, pi in enumerate(plist):
                        _, q, col = p.passes[pi]
                        Q = p.quotas[q]
                        col_l = col - (w0 * Q) // 128
                        pl = pass_ctr[q]
                        pass_ctr[q] += 1
                        nc.tensor.matmul(
                            ps1[:, :],
                            ms[q][:, col_l, :],
                            vh[q][:, :, pl],
                            start=(k == 0),
                            stop=(k == len(plist) - 1),
                        )
                    ag = aggxp.tile([D, p.w_win], f16i, tag="ag")
                    nc.scalar.copy(ag[:], ps1[:])
                    if debug_stage < 5:
                        continue
                    ps2 = ps2p.tile([p.w_win, D], f32, tag="ps2")
                    nc.tensor.matmul(
                        ps2[:, :],
                        ag[:, :],
                        wt16[:, :],
                        start=True,
                        stop=not p.bias_nonzero,
                    )
                    if p.bias_nonzero:
                        nc.tensor.matmul(
                            ps2[:, :],
                            sbrow16[:, w * p.w_win : (w + 1) * p.w_win],
                            b16[:, :],
                            start=False,
                            stop=True,
                        )
                    nc.scalar.activation(
                        out_sb[:, wl, :],
                        ps2[:, :],
                        mybir.ActivationFunctionType.Relu,
                        scale=sd_sb[:, w : w + 1],
                    )
                nc.sync.dma_start(out_d[:, w0 : w0 + gs, :], out_sb[:])
                ic_base += gidx_cols
                pass_base += gpass
    nc.compile()
    return nc


def _unshard(p, outs):
    N, D = p.N, p.D
    res = np.empty((N, D), dtype=np.float32)
    for c in range(p.n_cores):
        # [w_win, nwin, D] partition-major -> window-ordered rows
        o = np.asarray(outs[c]).transpose(1, 0, 2).reshape(p.nwin * p.w_win, D)
        memb = p.win_members[c]
        real = memb >= 0
        res[c * p.npc + memb[real]] = o[real]
    return res


def kernel(x, edge_index, W, b):
    from concourse.bass_utils import run_bass_kernel_spmd

    x = np.asarray(x, dtype=np.float32)
    W = np.asarray(W, dtype=np.float32)
    b = np.asarray(b, dtype=np.float32)
    p = prepare(x, edge_index, W, b)
    nc = build_program(p)
    in_maps = []
    for c in range(p.n_cores):
        m = {
            "x": x,
            "xself": p.xself_perm[c],
            "wt": p.WT,
            "iota": p.iota,
            "idx": p.idx_all[c],
            "doff": p.doff_all[c],
            "disv": p.disv_all[c],
            "sd": p.sd[c],
        }
        if p.bias_nonzero:
            m["sb"] = p.sb[c]
            m["b"] = p.b.reshape(1, -1)
        in_maps.append(m)
    res = run_bass_kernel_spmd(nc, in_maps, core_ids=list(range(p.n_cores)))
    outs = [r["out"] for r in res.results]
    return _unshard(p, outs)


# revision 28
# speedup vs baseline: 1.1983x; 1.1983x over previous
"""GCN message-passing kernel for Trainium2 (8 NeuronCores, Bass/Tile).

out = coef * relu(C_U * D^-1/2 A~^T D^-1/2 (x W^T + b)),  A~ = A + I

Strategy (dst-sharded, fully static SPMD program):
- Core c owns a 12,500-node dst range. Host deals dsts into 64-wide
  "windows" (degree-sorted round-robin for load balance), buckets each
  core's edges by (window, src-segment), pads to static per-(w,seg)
  quotas (max over cores/windows) so all 8 cores run one program.
- W commutes with aggregation: aggregate xs = dis_src*x rows first,
  apply W once per output node afterwards.
- Device: dma_gather pulls x rows (512B) from HBM by int16 index
  (4 src segments of 32768 rows + a per-core self-loop table); ACT
  casts msgs fp32->fp16; DVE builds per-pass "value-hot" [e,64]
  matrices (iota==dstoff)*dis_src in fp16; PE contracts
  msgs^T @ vh into PSUM [128=D, 64=dst] per window; stage-2 matmul
  applies W^T; ACT fuses relu + coef*C_U*dis_dst scale; DMA out.
- Host unpermutes the window-ordered output rows.
"""

import sys
import types

import numpy as np


def _install_ntff_hook_bridge():
    """antenv.axon_hooks is missing from this image; bridge it so
    run_bass_kernel_spmd(trace=True) can profile. Harmless if unused."""
    if "antenv.axon_hooks" in sys.modules:
        return
    hooks = types.ModuleType("antenv.axon_hooks")
    hooks._HOOK = None

    def _get():
        if hooks._HOOK is None:
            try:
                from trn_agent_boot.trn_boot import _ntff_profile_via_ctypes

                hooks._HOOK = _ntff_profile_via_ctypes("/opt/axon/libaxon_pjrt.so")
            except Exception:
                hooks._HOOK = None
        return hooks._HOOK

    hooks.get_axon_ntff_profile_hook = _get
    hooks.set_axon_ntff_profile_hook = lambda h: setattr(hooks, "_HOOK", h)
    sys.modules["antenv.axon_hooks"] = hooks


_install_ntff_hook_bridge()

C_SIGMA = 2.0
C_U = 1.0
SEG = 32768  # dma_gather int16 index reach
W_WIN = 64  # dst window width (one-hot width)
N_CORES = 8


def _ceil(a, b):
    return (a + b - 1) // b


def _wrap16(idx, ncols):
    """[n] int16 -> [128, ncols] wrapped in 16 partitions, replicated x8."""
    n = idx.shape[0]
    out = np.zeros((16, ncols), dtype=np.int16)
    out[np.arange(n) % 16, np.arange(n) // 16] = idx
    return np.tile(out, (8, 1))


def _slotwrap(vals, ncols, dtype):
    """[n] -> [128, ncols] with slot i at [i%128, i//128]."""
    n = vals.shape[0]
    out = np.zeros((128, ncols), dtype=dtype)
    out[np.arange(n) % 128, np.arange(n) // 128] = vals
    return out


class _Prep:
    """Host-side sharding/preprocessing result."""


def prepare(x, edge_index, W, b, n_cores=N_CORES, w_win=W_WIN, group=8):
    f16 = np.float16
    N, D = x.shape
    assert N % n_cores == 0
    npc = N // n_cores
    nwin = _ceil(npc, w_win)
    # make nwin a multiple of 4 so groups of 8 tile as 8,...,8,4
    nwin = _ceil(nwin, 4) * 4
    nseg = _ceil(N, SEG)

    src = np.asarray(edge_index[0], dtype=np.int64)
    dst = np.asarray(edge_index[1], dtype=np.int64)
    deg = np.bincount(src, minlength=N).astype(np.float32) + 1.0
    dis = deg ** -0.5  # float32

    p = _Prep()
    p.N, p.D, p.npc, p.nwin, p.nseg = N, D, npc, nwin, nseg
    p.n_cores, p.w_win, p.group = n_cores, w_win, group
    p.coef = np.sqrt(C_SIGMA / D).astype(np.float32)

    core_of = dst // npc
    dstloc = dst - core_of * npc

    # --- per-core window assignment: deal dsts (sorted by degree desc)
    # round-robin over nwin windows -> balanced loads; permutation undone
    # on the host at the end.
    # per-(node, segment) in-degree for balanced window packing
    segdeg = np.zeros((N, nseg), dtype=np.int64)
    np.add.at(segdeg, (dst, src // SEG), 1)

    p.win_members = []  # per core: [nwin*w_win] node-local ids (or -1 pad)
    p.win_of = np.empty((n_cores, npc), dtype=np.int32)
    p.pos_of = np.empty((n_cores, npc), dtype=np.int32)
    for c in range(n_cores):
        sd = segdeg[c * npc : (c + 1) * npc]  # [npc, nseg]
        tot = sd.sum(axis=1)
        order = np.argsort(-tot, kind="stable").astype(np.int32)
        # LPT vector bin-packing: heaviest dst first, place into the
        # window minimizing the resulting max per-segment load, with a
        # hard cap of w_win members per window.
        loads = np.zeros((nwin, nseg), dtype=np.float64)
        counts = np.zeros(nwin, dtype=np.int64)
        memb = -np.ones(nwin * w_win, dtype=np.int64)
        full_pen = np.zeros(nwin)
        for d in order:
            cand = (loads + sd[d]).max(axis=1) + full_pen
            w = int(np.argmin(cand))
            r = counts[w]
            counts[w] = r + 1
            if counts[w] >= w_win:
                full_pen[w] = 1e18
            loads[w] += sd[d]
            p.win_of[c, d] = w
            p.pos_of[c, d] = r
            memb[w * w_win + r] = d
        p.win_members.append(memb)

    # --- per-edge: core, window, dstoff, segment
    e_w = p.win_of[core_of, dstloc]
    e_off = p.pos_of[core_of, dstloc]
    e_q = (src // SEG).astype(np.int64)

    # --- quotas: max over (core, window) per segment, rounded to 32
    cnt = np.zeros((n_cores, nwin, nseg), dtype=np.int64)
    np.add.at(cnt, (core_of, e_w, e_q), 1)
    quotas = [max(32, int(_ceil(cnt[:, :, q].max(), 32) * 32)) for q in range(nseg)]
    p.quotas = quotas + [w_win]  # last stream = self loops
    p.nstream = nseg + 1
    p.cnt = cnt

    # --- static pass schedule (same for all cores)
    # stream q slots: window w occupies [w*Q, (w+1)*Q); 128-slot columns;
    # a column overlapping two windows is processed once per window.
    p.passes = []  # list of (w, q, col) in window-major emission order
    p.win_passes = [[] for _ in range(nwin)]  # per window: pass indices
    for w in range(nwin):
        for q in range(p.nstream):
            Q = p.quotas[q]
            c0 = (w * Q) // 128
            c1 = ((w + 1) * Q - 1) // 128
            for col in range(c0, c1 + 1):
                p.win_passes[w].append(len(p.passes))
                p.passes.append((w, q, col))
    p.npass = len(p.passes)

    # group layout
    p.ngroups = _ceil(nwin, group)
    p.group_sizes = [min(group, nwin - g * group) for g in range(p.ngroups)]
    assert all(gs in (group, 4) or gs == group for gs in p.group_sizes)

    # per-(group, stream): column count and pass count (static)
    p.gcols = [[gs * Q // 128 for Q in p.quotas] for gs in p.group_sizes]
    for row in p.gcols:
        for v in row:
            assert v * 128 % 128 == 0
    p.gpasses = []  # per group, per stream: number of passes
    for g, gs in enumerate(p.group_sizes):
        row = []
        for q in range(p.nstream):
            n = 0
            for w in range(g * group, g * group + gs):
                Q = p.quotas[q]
                n += ((w + 1) * Q - 1) // 128 - (w * Q) // 128 + 1
            row.append(n)
        p.gpasses.append(row)
    p.colsmax = max(max(row) for row in p.gpasses)

    # --- per-core slot fill + per-pass metadata
    p.idx_all = []  # [128, sum_g sum_q G*Q/16] int16
    p.doff_all = []  # [128, total_passes] f16
    p.disv_all = []  # [128, total_passes] f16
    p.sd = []
    tot_idx_cols = sum(
        sum(gs * Q // 16 for Q in p.quotas) for gs in p.group_sizes
    )
    tot_pass = sum(sum(row) for row in p.gpasses)
    p.tot_idx_cols, p.tot_pass = tot_idx_cols, tot_pass

    for c in range(n_cores):
        m = core_of == c
        cw, coff, cq, csrc = e_w[m], e_off[m], e_q[m], src[m]
        memb = p.win_members[c]

        # slot arrays per stream
        sl_idx = []
        sl_doff = []
        sl_dis = []
        for q in range(nseg):
            Q = p.quotas[q]
            S = nwin * Q
            nrows_q = min(N, (q + 1) * SEG) - q * SEG
            # spread pad reads across the segment (identical pad indices
            # serialize on one DRAM row: measured 3.6x slower)
            idx16 = ((np.arange(S, dtype=np.int64) * 7919) % nrows_q).astype(
                np.int16
            )
            doffv = -np.ones(S, dtype=np.float32)
            disv = np.zeros(S, dtype=np.float32)
            mq = cq == q
            wq, offq, srcq = cw[mq], coff[mq], csrc[mq]
            o = np.lexsort((srcq, wq))  # window-major, src-sorted within
            wq, offq, srcq = wq[o], offq[o], srcq[o]
            wcnt = np.bincount(wq, minlength=nwin)
            starts = np.concatenate([[0], np.cumsum(wcnt)[:-1]])
            rank = np.arange(len(wq)) - starts[wq]
            slot = wq * Q + rank
            idx16[slot] = (srcq - q * SEG).astype(np.int16)
            doffv[slot] = offq
            disv[slot] = dis[srcq]
            sl_idx.append(idx16)
            sl_doff.append(doffv)
            sl_dis.append(disv)
        # self stream: host pre-permutes the core's x slice into wrapped
        # window order -> plain sequential DMA on device (no gather).
        Q = w_win
        S = nwin * Q
        idx16 = np.zeros(S, dtype=np.int16)  # unused (sequential stream)
        doffv = -np.ones(S, dtype=np.float32)
        disv = np.zeros(S, dtype=np.float32)
        real = memb >= 0
        slots = np.arange(nwin * w_win)[real]
        nodes = memb[real]
        doffv[slots] = slots % w_win
        disv[slots] = dis[c * npc + nodes]
        sl_idx.append(idx16)
        sl_doff.append(doffv)
        sl_dis.append(disv)
        # wrapped layout [128, S/128, D]: slot i at [i%128, i//128]
        xsp = np.zeros((128, S // 128, D), dtype=np.float32)
        xsp[slots % 128, slots // 128] = x[c * npc + nodes]
        p.xself_perm = getattr(p, "xself_perm", [])
        p.xself_perm.append(xsp)

        # pack group-major: idx wrapped; passes built from slot arrays
        idx_cols = np.zeros((128, tot_idx_cols), dtype=np.int16)
        doff_cols = np.zeros((128, tot_pass), dtype=f16)
        disv_cols = np.zeros((128, tot_pass), dtype=f16)
        ic = 0
        pc_i = 0
        for g, gs in enumerate(p.group_sizes):
            w0 = g * group
            for q in range(p.nstream):
                Q = p.quotas[q]
                seg_slots = sl_idx[q][w0 * Q : (w0 + gs) * Q]
                ncol = gs * Q // 16
                idx_cols[:, ic : ic + ncol] = _wrap16(seg_slots, ncol)
                ic += ncol
            # stream-major within group (matches device vh-tile layout)
            for q in range(p.nstream):
                Q = p.quotas[q]
                for w in range(w0, w0 + gs):
                    c0 = (w * Q) // 128
                    c1 = ((w + 1) * Q - 1) // 128
                    for col in range(c0, c1 + 1):
                        s0 = col * 128
                        sl = np.arange(s0, s0 + 128)
                        inw = (sl >= w * Q) & (sl < (w + 1) * Q)
                        dv = np.full(128, -1.0, dtype=np.float32)
                        vv = np.zeros(128, dtype=np.float32)
                        gsl = sl[inw]
                        dv[inw] = sl_doff[q][gsl]
                        vv[inw] = sl_dis[q][gsl]
                        doff_cols[:, pc_i] = dv.astype(f16)
                        disv_cols[:, pc_i] = vv.astype(f16)
                        pc_i += 1
        assert pc_i == tot_pass, (pc_i, tot_pass)
        assert ic == tot_idx_cols
        p.idx_all.append(idx_cols)
        p.doff_all.append(doff_cols)
        p.disv_all.append(disv_cols)


        sdv = np.zeros((w_win, nwin), dtype=np.float32)
        nodes_per_win = memb.reshape(nwin, w_win)
        for w in range(nwin):
            mm = nodes_per_win[w] >= 0
            sdv[mm, w] = (
                p.coef * C_U * dis[c * npc + nodes_per_win[w][mm]]
            ).astype(np.float32)
        p.sd.append(sdv)

    # iota-expanded constant [128, w_win, colsmax] f16: value j at (p, j, c)
    io = np.broadcast_to(
        np.arange(w_win, dtype=np.float32)[None, :, None], (128, w_win, p.colsmax)
    )
    p.iota = np.ascontiguousarray(io.astype(f16))
    p.WT = np.ascontiguousarray(np.asarray(W, dtype=np.float32).T)
    p.b = np.asarray(b, dtype=np.float32)
    p.bias_nonzero = bool(np.any(p.b != 0))
    if p.bias_nonzero:
        # S_d = sum over incoming edges (incl self) of dis_src, laid out
        # as a row [1, nwin*w_win]: element w*w_win + i = window w, row i
        sb = np.zeros((n_cores, nwin * w_win), dtype=np.float32)
        np.add.at(sb, (core_of, e_w * w_win + e_off), dis[src])
        for c in range(n_cores):
            memb = p.win_members[c]
            real = memb >= 0
            slots = np.arange(nwin * w_win)[real]
            sb[c, slots] += dis[c * npc + memb[real]]
        p.sb = sb.reshape(n_cores, 1, nwin * w_win)
    return p


def build_program(p, debug_stage=5):
    import concourse.bacc as bacc
    import concourse.mybir as mybir
    import concourse.tile as tile

    f32, f16i, i16 = mybir.dt.float32, mybir.dt.float16, mybir.dt.int16
    D, nwin, group = p.D, p.nwin, p.group
    nstream = p.nstream

    nc = bacc.Bacc(
        "TRN2", target_bir_lowering=False, debug=False, num_swdge_queues=4
    )
    x_d = nc.dram_tensor("x", [p.N, D], f32, kind="ExternalInput")
    xself_d = nc.dram_tensor(
        "xself", [128, p.nwin * p.w_win // 128, D], f32, kind="ExternalInput"
    )
    wt_d = nc.dram_tensor("wt", [D, D], f32, kind="ExternalInput")
    iota_d = nc.dram_tensor("iota", [128, p.w_win, p.colsmax], f16i, kind="ExternalInput")
    idx_d = nc.dram_tensor("idx", [128, p.tot_idx_cols], i16, kind="ExternalInput")
    doff_d = nc.dram_tensor("doff", [128, p.tot_pass], f16i, kind="ExternalInput")
    disv_d = nc.dram_tensor("disv", [128, p.tot_pass], f16i, kind="ExternalInput")
    sd_d = nc.dram_tensor("sd", [p.w_win, nwin], f32, kind="ExternalInput")
    if p.bias_nonzero:
        sb_d = nc.dram_tensor("sb", [1, nwin * p.w_win], f32, kind="ExternalInput")
        b_d = nc.dram_tensor("b", [1, D], f32, kind="ExternalInput")
    # partition-major layout: out[i, w, :] = window w, row i (host unshards)
    out_d = nc.dram_tensor("out", [p.w_win, nwin, D], f32, kind="ExternalOutput")

    segs = []  # (dram AP, nrows) per gather stream
    for q in range(p.nseg):
        lo = q * SEG
        hi = min(p.N, lo + SEG)
        segs.append(x_d[lo:hi, :])
    segs.append(None)  # self stream: pre-permuted, sequential DMA

    with tile.TileContext(nc) as tc:
        with (
            tc.tile_pool(name="const", bufs=1) as constp,
            tc.tile_pool(name="meta", bufs=2) as metap,
            tc.tile_pool(name="gbuf", bufs=2) as gbufp,
            tc.tile_pool(name="msgs", bufs=2) as msgsp,
            tc.tile_pool(name="vh", bufs=2) as vhp,
            tc.tile_pool(name="aggx", bufs=3) as aggxp,
            tc.tile_pool(name="outsb", bufs=2) as outp,
            tc.tile_pool(name="ps1", bufs=4, space="PSUM") as ps1p,
            tc.tile_pool(name="ps2", bufs=2, space="PSUM") as ps2p,
        ):
            # constants
            wt32 = constp.tile([D, D], f32, tag="wt32")
            nc.sync.dma_start(wt32[:], wt_d[:])
            wt16 = constp.tile([D, D], f16i, tag="wt16")
            nc.scalar.copy(wt16[:], wt32[:])
            iota_sb = constp.tile([128, p.w_win, p.colsmax], f16i, tag="iota")
            nc.sync.dma_start(iota_sb[:], iota_d[:])
            sd_sb = constp.tile([p.w_win, nwin], f32, tag="sd")
            nc.sync.dma_start(sd_sb[:], sd_d[:])
            if p.bias_nonzero:
                sb_sb = constp.tile([1, nwin * p.w_win], f32, tag="sb")
                nc.sync.dma_start(sb_sb[:], sb_d[:])
                b32 = constp.tile([1, D], f32, tag="b32")
                nc.sync.dma_start(b32[:], b_d[:])
                b16 = constp.tile([1, D], f16i, tag="b16")
                nc.scalar.copy(b16[:], b32[:])
                sbrow16 = constp.tile([1, nwin * p.w_win], f16i, tag="sbw16")
                nc.scalar.copy(sbrow16[:], sb_sb[:])

            ic_base = 0
            pass_base = 0
            gq = [0]  # round-robin SWDGE queue counter (mutable closure)
            for g, gs in enumerate(p.group_sizes):
                w0 = g * group
                gidx_cols = sum(gs * Q // 16 for Q in p.quotas)
                gpass = sum(p.gpasses[g])
                idx_sb = metap.tile([128, gidx_cols], i16, tag="idx")
                nc.sync.dma_start(idx_sb[:], idx_d[:, ic_base : ic_base + gidx_cols])
                doff_sb = metap.tile([128, gpass], f16i, tag="doff")
                nc.sync.dma_start(doff_sb[:], doff_d[:, pass_base : pass_base + gpass])
                disv_sb = metap.tile([128, gpass], f16i, tag="disv")
                nc.sync.dma_start(disv_sb[:], disv_d[:, pass_base : pass_base + gpass])

                # gathers + cast + vh build, per stream
                gb = []
                ms = []
                vh = []
                icol = 0
                ppos = 0
                for q in range(nstream):
                    Q = p.quotas[q]
                    ncols = gs * Q // 128
                    nix = gs * Q // 16
                    npas = p.gpasses[g][q]
                    gt = gbufp.tile([128, ncols, D], f32, tag=f"g{q}")
                    if segs[q] is None:
                        # self stream: sequential slice of the pre-permuted
                        # per-core table
                        c0 = w0 * Q // 128
                        nc.sync.dma_start(gt[:], xself_d[:, c0 : c0 + ncols, :])
                    else:
                        # SWDGE ring holds 64 descs/lane -> max 1024 idxs
                        # per gather; round-robin the 4 queues (4 Q7 pairs
                        # generate in parallel).
                        off = 0
                        total = gs * Q
                        while off < total:
                            n = min(1024, total - off)
                            nc.gpsimd.dma_gather(
                                gt[:, off // 128 : (off + n) // 128, :],
                                segs[q],
                                idx_sb[:, icol + off // 16 : icol + (off + n) // 16],
                                n,
                                n,
                                D,
                                queue_num=gq[0] % 4,
                            )
                            gq[0] += 1
                            off += n
                    mt = msgsp.tile([128, ncols, D], f16i, tag=f"m{q}")
                    if debug_stage >= 2:
                        nc.scalar.copy(mt[:], gt[:])
                    vt = vhp.tile([128, p.w_win, npas], f16i, tag=f"v{q}")

                    def _bcast(ap2d, n=npas):
                        return ap2d.rearrange("p (o c) -> p o c", o=1).broadcast_to(
                            [128, p.w_win, n]
                        )

                    if debug_stage >= 3:
                        nc.vector.tensor_tensor(
                            vt[:],
                            iota_sb[:, :, :npas],
                            _bcast(doff_sb[:, ppos : ppos + npas]),
                            mybir.AluOpType.is_equal,
                        )
                        nc.vector.tensor_tensor(
                            vt[:],
                            vt[:],
                            _bcast(disv_sb[:, ppos : ppos + npas]),
                            mybir.AluOpType.mult,
                        )
                    gb.append(gt)
                    ms.append(mt)
                    vh.append(vt)
                    icol += nix
                    ppos += npas

                out_sb = outp.tile([p.w_win, gs, D], f32, tag="out")
                if debug_stage < 5:
                    nc.vector.memset(out_sb[:], 0.0)
                # windows
                pass_ctr = [0] * nstream
                for wl in range(gs):
                    w = w0 + wl
                    if debug_stage < 4:
                        continue
                    ps1 = ps1p.tile([D, p.w_win], f32, tag="ps1")
                    plist = p.win_passes[w]
                    for k, pi in enumerate(plist):
                        _, q, col = p.passes[pi]
                        Q = p.quotas[q]
                        col_l = col - (w0 * Q) // 128
                        pl = pass_ctr[q]
                        pass_ctr[q] += 1
                        nc.tensor.matmul(
                            ps1[:, :],
                            ms[q][:, col_l, :],
                            vh[q][:, :, pl],
                            start=(k == 0),
                            stop=(k == len(plist) - 1),
                        )
                    ag = aggxp.tile([D, p.w_win], f16i, tag="ag")
                    nc.scalar.copy(ag[:], ps1[:])
                    if debug_stage < 5:
                        continue
                    ps2 = ps2p.tile([p.w_win, D], f32, tag="ps2")
                    nc.tensor.matmul(
                        ps2[:, :],
                        ag[:, :],
                        wt16[:, :],
                        start=True,
                        stop=not p.bias_nonzero,
                    )
                    if p.bias_nonzero:
                        nc.tensor.matmul(
                            ps2[:, :],
                            sbrow16[:, w * p.w_win : (w + 1) * p.w_win],
                            b16[:, :],
                            start=False,
                            stop=True,
                        )
                    nc.scalar.activation(
                        out_sb[:, wl, :],
                        ps2[:, :],
                        mybir.ActivationFunctionType.Relu,
                        scale=sd_sb[:, w : w + 1],
                    )
                nc.sync.dma_start(out_d[:, w0 : w0 + gs, :], out_sb[:])
                ic_base += gidx_cols
                pass_base += gpass
    nc.compile()
    return nc


def _unshard(p, outs):
    N, D = p.N, p.D
    res = np.empty((N, D), dtype=np.float32)
    for c in range(p.n_cores):
        # [w_win, nwin, D] partition-major -> window-ordered rows
        o = np.asarray(outs[c]).transpose(1, 0, 2).reshape(p.nwin * p.w_win, D)
        memb = p.win_members[c]
        real = memb >= 0
        res[c * p.npc + memb[real]] = o[real]
    return res


def kernel(x, edge_index, W, b):
    from concourse.bass_utils import run_bass_kernel_spmd

    x = np.asarray(x, dtype=np.float32)
    W = np.asarray(W, dtype=np.float32)
    b = np.asarray(b, dtype=np.float32)
    p = prepare(x, edge_index, W, b)
    nc = build_program(p)
    in_maps = []
    for c in range(p.n_cores):
        m = {
            "x": x,
            "xself": p.xself_perm[c],
            "wt": p.WT,
            "iota": p.iota,
            "idx": p.idx_all[c],
            "doff": p.doff_all[c],
            "disv": p.disv_all[c],
            "sd": p.sd[c],
        }
        if p.bias_nonzero:
            m["sb"] = p.sb[c]
            m["b"] = p.b.reshape(1, -1)
        in_maps.append(m)
    res = run_bass_kernel_spmd(nc, in_maps, core_ids=list(range(p.n_cores)))
    outs = [r["out"] for r in res.results]
    return _unshard(p, outs)
